# revision 19
# baseline (speedup 1.0000x reference)
"""MoE-GRN kernel for Trainium2, 8 NeuronCores, data-parallel over batch,
with on-device top-2 routing so fc2 only computes selected experts.

Reference (B=4096, IN=1024, J=HID*E=16384, Dtot=OUT*E=8192, E=8, C=1000, K=2):
    gate_logits = x @ Wg.T + bg                     [B, E]
    Gx = ||gate_logits||_2 per row; Nx = Gx / (mean_B(Gx) + 1e-6)
    gate_probs = softmax(gamma * (gate_logits * Nx) + beta)
    top2 over E=8
    h  = relu(x @ W1.T + b1)                        [B, J]
    eo = (h @ W2.T + b2).reshape(B, E, OUT)
    out = sum_k topk_probs * eo[topk_idx]           [B, OUT]
    y  = out @ Wc.T + bc                            [B, C]

Only 2 of 8 expert output blocks are needed per token, so the dense fc2
(B*J*Dtot MACs) is 4x oversized.  Per core (512 tokens): compute probs +
top-2 mask exactly (fp32 gating, min top2/top3 gap ~4e-5), compact each
expert's token list with gpsimd.sparse_gather (capacity 160 per expert,
observed max 153), ap_gather the fp16 h columns per expert per J-split,
run fc2 on just those columns, then weight by gate probs and scatter_add
back into a token-indexed buffer (bf16 hi+lo split keeps fp32-ish
precision through the bf16-only scatter path).

Token "slots": gating subtile st / partition p computes the token at xs
column p*4+st (host permutes the gating copy xg accordingly) so that the
(p, st) value grid folds into sparse_gather's 16-partition-wrapped stream
order with a pure strided DRAM readback.  Invalid (padding) list entries
are pointed at trash slot 512 whose gate weight is 0.

Precision: fc1/fc2 run in fp16 (weights + activations; moving-operand
fp16 keeps full PE rate at the 160-wide routed matmuls, and fp16's 11-bit
mantissa keeps the error well inside the 2e-2 gate where bf16 fails);
gating is fp32; combine/classifier fp16/f32.
"""

import numpy as np
import ml_dtypes

import concourse.bass as bass
import concourse.mybir as mybir
import concourse.tile as tile
from concourse import bacc
from concourse.bass_utils import run_bass_kernel_spmd

F32 = mybir.dt.float32
F32R = mybir.dt.float32r
F16 = mybir.dt.float16
BF16 = mybir.dt.bfloat16
I16 = mybir.dt.int16
U32 = mybir.dt.uint32
AF = mybir.ActivationFunctionType
ALU = mybir.AluOpType

B, IN, J, DTOT, E, C = 4096, 1024, 16384, 8192, 8, 1000
NCORES = 8
T = B // NCORES            # 512 tokens per core
TS = T // 128              # 4 token subtiles
ITS = IN // 128            # 8 k-subtiles over IN
NSPLIT = 8                 # J split into 8 chunks of 2048
JT_PER_S = J // NSPLIT // 128   # 16 j-tiles per split
OT = 1024 // 128           # 8 out-subtiles per expert block
NDT = DTOT // 128          # 64 d-tiles
NTT = B // 128             # 32 full-batch token tiles
CE = 160                   # routed capacity per (core, expert)
CW = CE // 16              # capacity in 16-wrapped columns
NSLOT = T + 1              # 512 real slots + trash slot 512
EPS = 1e-6


def _build(flags):
    has_bg, has_gb, has_b2, has_bc = (
        flags["bg"], flags["gb"], flags["b2"], flags["bc"])
    nc = bacc.Bacc("TRN2", target_bir_lowering=False)

    # ---- DRAM I/O ----
    xf_d = nc.dram_tensor("xf", [NTT, 128, ITS, 128], BF16, kind="ExternalInput")
    xg_d = nc.dram_tensor("xg", [128, ITS, T], F32, kind="ExternalInput")
    xs_d = nc.dram_tensor("xs", [128, ITS, T], F16, kind="ExternalInput")
    wg_d = nc.dram_tensor("wg", [128, ITS, E], F32, kind="ExternalInput")
    wgh_d = nc.dram_tensor("wgh", [128, ITS, E], BF16, kind="ExternalInput")
    w1_d = nc.dram_tensor("w1", [J // 128, 128, ITS, 128], F16, kind="ExternalInput")
    w2_d = nc.dram_tensor("w2", [NSPLIT, E, OT, 128, JT_PER_S, 128], F16,
                          kind="ExternalInput")
    wc_d = nc.dram_tensor("wc", [128, OT, C], F16, kind="ExternalInput")
    b1_d = nc.dram_tensor("b1s", [128, J // 128], F32, kind="ExternalInput")
    cidx_d = nc.dram_tensor("cidx", [16, CW], F32, kind="ExternalInput")
    slotp1_d = nc.dram_tensor("slotp1", [128, TS, E], F32, kind="ExternalInput")
    if has_bg:
        bg_d = nc.dram_tensor("bgb", [128, E], F32, kind="ExternalInput")
        bgh_d = nc.dram_tensor("bgbh", [128, E], BF16, kind="ExternalInput")
    if has_gb:
        ga_d = nc.dram_tensor("gammab", [128, E], F32, kind="ExternalInput")
        be_d = nc.dram_tensor("betab", [128, E], F32, kind="ExternalInput")
    if has_b2:
        b2_d = nc.dram_tensor("b2s", [128, NDT], F32, kind="ExternalInput")
    if has_bc:
        bc_d = nc.dram_tensor("bcr", [1, C], F16, kind="ExternalInput")
    out_d = nc.dram_tensor("out", [TS, 128, C], F32, kind="ExternalOutput")

    with tile.TileContext(nc) as tc:
        with tc.tile_pool(name="const", bufs=1) as cp, \
             tc.tile_pool(name="dram", bufs=1, space="DRAM") as dp, \
             tc.tile_pool(name="ps", bufs=6, space="PSUM") as psp:
            # resident tiles needed by fc1 first (their DMAs land first)
            xs = cp.tile([128, ITS, T], F16, tag="xs")
            nc.sync.dma_start(xs[:], xs_d[:])
            b1s = cp.tile([128, J // 128], F32, tag="b1s")
            nc.sync.dma_start(b1s[:], b1_d[:])
            split_cm = [tc.tile_pool(name="hqp", bufs=2),
                        tc.tile_pool(name="w1p", bufs=3),
                        tc.tile_pool(name="w2p", bufs=5),
                        tc.tile_pool(name="gthp", bufs=2)]
            hqp, w1p, w2p, gthp = [cm.__enter__() for cm in split_cm]

            def fc1_split(s):
                # hq token-major fp16 [slot, jtl]; slot 512 zeroed (trash)
                hq = hqp.tile([128, NSLOT, JT_PER_S], F16, tag="hq")
                nc.any.memset(hq[:, T, :], 0.0)
                for jtl in range(JT_PER_S):
                    jt = s * JT_PER_S + jtl
                    w1t = w1p.tile([128, ITS, 128], F16, tag="w1t")
                    nc.sync.dma_start(w1t[:], w1_d[jt])
                    ph = psp.tile([128, T], F32, tag="ps_shared")
                    for it in range(ITS):
                        nc.tensor.matmul(ph[:], w1t[:, it, :], xs[:, it, :],
                                         start=(it == 0), stop=(it == ITS - 1))
                    nc.scalar.activation(hq[:, 0:T, jtl], ph[:], AF.Relu,
                                         bias=b1s[:, jt:jt + 1])
                return hq

            # ---- split 0 fc1 first: PE has dense work from the start ----
            hq0 = fc1_split(0)

            # ---- gating ----
            wg = cp.tile([128, ITS, E], F32, tag="wg")
            nc.sync.dma_start(wg[:], wg_d[:])
            wgh = cp.tile([128, ITS, E], BF16, tag="wgh")
            nc.sync.dma_start(wgh[:], wgh_d[:])
            cidx = cp.tile([16, CW], F32, tag="cidx")
            nc.sync.dma_start(cidx[:], cidx_d[:])
            slotp1 = cp.tile([128, TS, E], F32, tag="slotp1")
            nc.sync.dma_start(slotp1[:], slotp1_d[:])
            if has_bg:
                bgb = cp.tile([128, E], F32, tag="bgb")
                nc.sync.dma_start(bgb[:], bg_d[:])
                bgbh = cp.tile([128, E], BF16, tag="bgbh")
                nc.sync.dma_start(bgbh[:], bgh_d[:])
            if has_gb:
                gab = cp.tile([128, E], F32, tag="gammab")
                nc.sync.dma_start(gab[:], ga_d[:])
                beb = cp.tile([128, E], F32, tag="betab")
                nc.sync.dma_start(beb[:], be_d[:])
            ones1 = cp.tile([1, 128], F32, tag="ones1")
            nc.any.memset(ones1[:], 1.0)
            ones_c = cp.tile([128, 1], F32, tag="ones_c")
            nc.any.memset(ones_c[:], 1.0)

            # slot-indexed per-expert gate weights (0 for unselected/trash)
            wb = cp.tile([128, E, NSLOT], F32, tag="wb")
            nc.any.memset(wb[:, :, T], 0.0)
            # routed idx lists: per expert CE entries, 16-wrapped, int16.
            # idx512 variant points invalid entries at trash slot 512.
            idx128 = cp.tile([128, E * CW], I16, tag="idx128")
            wcol = cp.tile([128, E, CE], F32, tag="wcol")
            # fc2 accumulator over splits + final combine target
            eoac = cp.tile([128, E * CE, OT], F32, tag="eoac")
            moeh = cp.tile([128, NSLOT, OT], BF16, tag="moeh")
            nc.any.memset(moeh[:], 0.0)
            moel = cp.tile([128, NSLOT, OT], BF16, tag="moel")
            nc.any.memset(moel[:], 0.0)

            gating_cm = [tc.tile_pool(name="gxp", bufs=1),
                         tc.tile_pool(name="gin", bufs=2),
                         tc.tile_pool(name="gtmp", bufs=4),
                         tc.tile_pool(name="gps", bufs=2, space="PSUM")]
            gxp, gin, gt, gps = [cm.__enter__() for cm in gating_cm]
            if True:
                xg = gxp.tile([128, ITS, T], F32, tag="xg")
                nc.sync.dma_start(xg[:], xg_d[:])
                ss_all = gxp.tile([128, NTT], F32, tag="ss_all")
                # full-batch squared row norms of gate logits (bf16 inputs:
                # only feeds the batch mean, which averages the error away)
                for tt in range(NTT):
                    xt = gin.tile([128, ITS, 128], BF16, tag="xf_t")
                    nc.sync.dma_start(xt[:], xf_d[tt])
                    pg = gps.tile([128, E], F32, tag="pg")
                    for it in range(ITS):
                        nc.tensor.matmul(pg[:], xt[:, it, :], wgh[:, it, :],
                                         start=(it == 0), stop=(it == ITS - 1))
                    if has_bg:
                        lg = gt.tile([128, E], F32, tag="lg")
                        nc.vector.tensor_add(lg[:], pg[:], bgbh[:])
                        src = lg
                    else:
                        src = pg
                    sq = gt.tile([128, E], F32, tag="sq")
                    nc.scalar.square(sq[:], src[:])
                    nc.vector.reduce_sum(ss_all[:, tt:tt + 1], sq[:],
                                         axis=mybir.AxisListType.X)
                gx_all = gt.tile([128, NTT], F32, tag="gx_all")
                nc.scalar.activation(gx_all[:], ss_all[:], AF.Sqrt)
                gsum = gt.tile([128, 1], F32, tag="gsum")
                nc.vector.reduce_sum(gsum[:], gx_all[:], axis=mybir.AxisListType.X)
                # partition-sum + mean + reciprocal + partition-broadcast, all
                # via tiny PE matmuls
                ptot = gps.tile([128, E], F32, tag="pg")
                nc.tensor.matmul(ptot[:1, :1], ones_c[:], gsum[:],
                                 start=True, stop=True)
                t1 = gt.tile([1, 1], F32, tag="t1")
                nc.vector.tensor_scalar(t1[:], ptot[:1, :1], 1.0 / B, EPS,
                                        op0=ALU.mult, op1=ALU.add)
                rec1 = gt.tile([1, 1], F32, tag="rec1")
                nc.vector.reciprocal(rec1[:], t1[:])
                pbc = gps.tile([128, E], F32, tag="pg")
                nc.tensor.matmul(pbc[:, :1], ones1[:], rec1[:],
                                 start=True, stop=True)
                nxs = gt.tile([128, 1], F32, tag="nxs")
                nc.scalar.copy(nxs[:], pbc[:, :1])

                # shard gating (fp32, exact) -> top2-masked prob weights.
                # gating column st*128+p is slot p*4+st (host permutes xg).
                w_all = gxp.tile([128, TS, E], F32, tag="w_all")
                for st in range(TS):
                    pgs = gps.tile([128, E], F32, tag="pg")
                    for it in range(ITS):
                        nc.tensor.matmul(pgs[:],
                                         xg[:, it, st * 128:(st + 1) * 128],
                                         wg[:, it, :],
                                         start=(it == 0), stop=(it == ITS - 1))
                    lgs = gt.tile([128, E], F32, tag="lgs")
                    if has_bg:
                        nc.vector.tensor_add(lgs[:], pgs[:], bgb[:])
                    else:
                        nc.scalar.copy(lgs[:], pgs[:])
                    sq = gt.tile([128, E], F32, tag="sq")
                    nc.scalar.square(sq[:], lgs[:])
                    ss1 = gt.tile([128, 1], F32, tag="ss1")
                    nc.vector.reduce_sum(ss1[:], sq[:], axis=mybir.AxisListType.X)
                    gx1 = gt.tile([128, 1], F32, tag="gx1")
                    nc.scalar.activation(gx1[:], ss1[:], AF.Sqrt)
                    nx = gt.tile([128, 1], F32, tag="nx")
                    nc.vector.tensor_mul(nx[:], gx1[:], nxs[:])
                    mod = gt.tile([128, E], F32, tag="mod")
                    nc.vector.tensor_scalar_mul(mod[:], lgs[:], nx[:])
                    if has_gb:
                        nc.vector.tensor_mul(mod[:], mod[:], gab[:])
                        nc.vector.tensor_add(mod[:], mod[:], beb[:])
                    rmax = gt.tile([128, 1], F32, tag="rmax")
                    nc.vector.reduce_max(rmax[:], mod[:], axis=mybir.AxisListType.X)
                    nrm = gt.tile([128, 1], F32, tag="nrm")
                    nc.vector.tensor_scalar_mul(nrm[:], rmax[:], -1.0)
                    ex = gt.tile([128, E], F32, tag="ex")
                    nc.scalar.activation(ex[:], mod[:], AF.Exp, bias=nrm[:])
                    sm = gt.tile([128, 1], F32, tag="sm")
                    nc.vector.reduce_sum(sm[:], ex[:], axis=mybir.AxisListType.X)
                    rs = gt.tile([128, 1], F32, tag="rs")
                    nc.vector.reciprocal(rs[:], sm[:])
                    probs = gt.tile([128, E], F32, tag="probs")
                    nc.vector.tensor_scalar_mul(probs[:], ex[:], rs[:])
                    mx8 = gt.tile([128, 8], F32, tag="mx8")
                    nc.vector.max(mx8[:], probs[:])
                    msk = gt.tile([128, E], F32, tag="msk")
                    nc.vector.tensor_scalar(msk[:], probs[:], mx8[:, 1:2], None,
                                            op0=ALU.is_ge)
                    nc.vector.tensor_mul(w_all[:, st, :], msk[:], probs[:])

                # ---- compaction: per-expert routed token lists ----
                # val[p, st, e] = slot (p*4+st) if expert selected else -1
                m01 = gt.tile([128, TS, E], F32, tag="m01")
                nc.vector.tensor_scalar(m01[:], w_all[:], 0.0, None,
                                        op0=ALU.is_gt)
                val = gxp.tile([128, TS, E], F32, tag="val")
                nc.vector.tensor_mul(val[:], m01[:], slotp1[:])
                nc.vector.tensor_scalar(val[:], val[:], -1.0, None,
                                        op0=ALU.add)
                # bounce both val and w through DRAM to reach slot-major
                # 16-wrap layouts (SWDGE queue, off the big DMA rings)
                vdr = dp.tile([128, TS, E], F32, tag="vdr")
                nc.gpsimd.dma_start(vdr[:], val[:])
                wdr = dp.tile([E, TS, 128], F32, tag="wdr")
                for st in range(TS):
                    nc.gpsimd.dma_start(wdr[:, st, :].rearrange("e p -> p e"),
                                        w_all[:, st, :])
                # wb[*, e, slot] with slot = p*4+st  <=  wdr[e, st, p]
                wrows = gxp.tile([1, E, T], F32, tag="wrows")
                for e in range(E):
                    nc.gpsimd.dma_start(
                        wrows[:, e, :].rearrange("o (p s) -> o p s", s=TS),
                        wdr[e].rearrange("s p -> p s")[None])
                    nc.gpsimd.partition_broadcast(wb[:, e, 0:T], wrows[:, e, :])

                vread = vdr[:].rearrange("(a b) s e -> e (b s) a", b=TS)
                for e in range(E):
                    vin = gin.tile([16, T // 16], F32, tag="vin")
                    nc.gpsimd.dma_start(vin[:], vread[e])
                    sg = gin.tile([16, CW], F32, tag="sg")
                    nf = gin.tile([1, 1], U32, tag="nf")
                    nc.gpsimd.sparse_gather(sg[:], vin[:], num_found=nf[:])
                    nff = gt.tile([1, 1], F32, tag="nff")
                    nc.vector.tensor_copy(nff[:], nf[:])
                    nf128 = gt.tile([128, 1], F32, tag="nf128")
                    nc.gpsimd.partition_broadcast(nf128[:], nff[:])
                    vmask = gt.tile([16, CW], F32, tag="vmask")
                    nc.vector.tensor_tensor(
                        vmask[:], cidx[:], nf128[0:16, :].to_broadcast([16, CW]),
                        op=ALU.is_lt)
                    # invalid entries -> trash slot 512, branchlessly:
                    # clamp(sg,0,T), then vmask*(sgc-T)+T
                    sgc = gt.tile([16, CW], F32, tag="sgc")
                    nc.vector.tensor_scalar(sgc[:], sg[:], 0.0, float(T),
                                            op0=ALU.max, op1=ALU.min)
                    nc.vector.tensor_scalar(sgc[:], sgc[:], float(T), None,
                                            op0=ALU.subtract)
                    sfix = gt.tile([16, CW], F32, tag="sfix")
                    nc.vector.tensor_tensor(sfix[:], vmask[:], sgc[:],
                                            op=ALU.mult)
                    nc.vector.tensor_scalar(sfix[:], sfix[:], float(T), None,
                                            op0=ALU.add)
                    nc.vector.tensor_copy(idx128[0:16, e * CW:(e + 1) * CW],
                                          sfix[:])
                # replicate idx lists to all 8 16-partition groups
                for g in range(1, 8):
                    nc.gpsimd.dma_start(idx128[16 * g:16 * g + 16, :],
                                        idx128[0:16, :])
                # per-slot gate weights for each expert's routed columns
                for e in range(E):
                    nc.gpsimd.ap_gather(
                        wcol[:, e, :], wb[:, e, :],
                        idx128[:, e * CW:(e + 1) * CW],
                        channels=128, num_elems=NSLOT, d=1, num_idxs=CE)
            for cm in reversed(gating_cm):
                cm.__exit__(None, None, None)

            # ---- fc2: routed per-expert matmuls, accumulated over splits ----
            if has_b2:
                b2s = cp.tile([128, NDT], F32, tag="b2s")
                nc.sync.dma_start(b2s[:], b2_d[:])

            def fc2_split(s, hq):
                for e in range(E):
                    gth = gthp.tile([128, CE, JT_PER_S], F16, tag="gth")
                    nc.gpsimd.ap_gather(
                        gth[:], hq[:],
                        idx128[:, e * CW:(e + 1) * CW],
                        channels=128, num_elems=NSLOT, d=JT_PER_S,
                        num_idxs=CE)
                    # transpose to kt-major so every fc2 matmul reads a
                    # CONTIGUOUS moving operand (strided rhs starves the PE)
                    gtt = gthp.tile([128, JT_PER_S, CE], F16, tag="gtt")
                    nc.vector.tensor_copy(gtt[:],
                                          gth[:].rearrange("p c k -> p k c"))
                    for ot in range(OT):
                        w2t = w2p.tile([128, JT_PER_S, 128], F16, tag="w2t")
                        nc.sync.dma_start(w2t[:], w2_d[s, e, ot])
                        pe_ = psp.tile([128, CE], F32, tag="ps_shared")
                        for ktl in range(JT_PER_S):
                            nc.tensor.matmul(pe_[:], w2t[:, ktl, :],
                                             gtt[:, ktl, :],
                                             start=(ktl == 0),
                                             stop=(ktl == JT_PER_S - 1))
                        seg = eoac[:, e * CE:(e + 1) * CE, ot]
                        if s == 0:
                            if has_b2:
                                nc.scalar.activation(
                                    seg, pe_[:], AF.Identity,
                                    bias=b2s[:, e * OT + ot:e * OT + ot + 1])
                            else:
                                nc.scalar.copy(seg, pe_[:])
                        else:
                            nc.vector.tensor_add(seg, seg, pe_[:])

            fc2_split(0, hq0)
            clp_cm = tc.tile_pool(name="clsp", bufs=1)
            clp = None
            for s in range(1, NSPLIT):
                if s == NSPLIT - 1:
                    # prefetch classifier weights behind the last split's w2
                    clp = clp_cm.__enter__()
                    wc = clp.tile([128, OT, C], F16, tag="wc")
                    nc.sync.dma_start(wc[:], wc_d[:])
                    if has_bc:
                        bct = clp.tile([1, C], F16, tag="bcr")
                        nc.sync.dma_start(bct[:], bc_d[:])
                        ones1h = clp.tile([1, 128], F16, tag="ones1h")
                        nc.any.memset(ones1h[:], 1.0)
                hq = fc1_split(s)
                fc2_split(s, hq)

            # ---- combine: weight by gate prob, bf16 hi/lo scatter-add ----
            with tc.tile_pool(name="cmb", bufs=1) as cmb:
                for e in range(E):
                    eow = cmb.tile([128, CE, OT], F32, tag="eow")
                    nc.vector.tensor_tensor(
                        eow[:], eoac[:, e * CE:(e + 1) * CE, :],
                        wcol[:, e, :].rearrange("p (n u) -> p n u", u=1)
                        .to_broadcast([128, CE, OT]),
                        op=ALU.mult)
                    ehi = cmb.tile([128, CE, OT], BF16, tag="ehi")
                    nc.vector.tensor_copy(ehi[:], eow[:])
                    elo = cmb.tile([128, CE, OT], BF16, tag="elo")
                    nc.vector.tensor_tensor(elo[:], eow[:], ehi[:],
                                            op=ALU.subtract)
                    nc.gpsimd.scatter_add(
                        moeh[:], idx128[:, e * CW:(e + 1) * CW], ehi[:],
                        channels=128, num_elems=NSLOT, d=OT, num_idxs=CE)
                    nc.gpsimd.scatter_add(
                        moel[:], idx128[:, e * CW:(e + 1) * CW], elo[:],
                        channels=128, num_elems=NSLOT, d=OT, num_idxs=CE)
                moe16 = clp.tile([128, NSLOT, OT], F16, tag="moe16")
                nc.vector.tensor_add(moe16[:], moeh[:], moel[:])

            # ---- classifier (fp16 x fp16) ----
            with tc.tile_pool(name="outp", bufs=2) as outp:
                for st in range(TS):
                    ot_ = outp.tile([128, C], F32, tag="ot")
                    for c0, cw_ in ((0, 512), (512, C - 512)):
                        pc = psp.tile([128, T], F32, tag="ps_shared")
                        for kt in range(OT):
                            nc.tensor.matmul(
                                pc[:, :cw_],
                                moe16[:, st * 128:(st + 1) * 128, kt],
                                wc[:, kt, c0:c0 + cw_],
                                start=(kt == 0),
                                stop=(kt == OT - 1 and not has_bc))
                        if has_bc:
                            nc.tensor.matmul(pc[:, :cw_], ones1h[:],
                                             bct[:, c0:c0 + cw_],
                                             start=False, stop=True)
                        nc.scalar.copy(ot_[:, c0:c0 + cw_], pc[:, :cw_])
                    nc.sync.dma_start(out_d[st], ot_[:])
            clp_cm.__exit__(None, None, None)
            for cm in reversed(split_cm):
                cm.__exit__(None, None, None)

    nc.compile()
    return nc


_CACHE = {}


def _get_program(flags):
    key = tuple(sorted(flags.items()))
    if key not in _CACHE:
        _CACHE[key] = _build(flags)
    return _CACHE[key]


def _prep_inputs(x, Wg, bg, gamma, beta, W1, b1, W2, b2, Wc, bc):
    f = np.float32
    bf = ml_dtypes.bfloat16
    f16 = np.float16
    a = np.ascontiguousarray
    x = np.asarray(x, f)
    flags = {
        "bg": bool(np.any(np.asarray(bg))),
        "gb": bool(np.any(np.asarray(gamma) != 1.0) or np.any(np.asarray(beta))),
        "b2": bool(np.any(np.asarray(b2))),
        "bc": bool(np.any(np.asarray(bc))),
    }
    wg_t = np.asarray(Wg, f).reshape(E, ITS, 128).transpose(2, 1, 0)
    # constants for on-device compaction
    cidx = (np.arange(CW)[None, :] * 16 + np.arange(16)[:, None]).astype(f)
    slotp1 = (np.arange(128)[:, None] * TS + np.arange(TS)[None, :] + 1.0)
    slotp1 = np.broadcast_to(slotp1[:, :, None], (128, TS, E)).astype(f)
    shared = {
        "xf": a(x.reshape(NTT, 128, ITS, 128).transpose(0, 3, 2, 1)
                .astype(bf)),
        "wg": a(wg_t),
        "wgh": a(wg_t.astype(bf)),
        "w1": a(np.asarray(W1, f).reshape(J // 128, 128, ITS, 128)
                .transpose(0, 3, 2, 1).astype(f16)),
        "w2": a(np.asarray(W2, f).reshape(E, OT, 128, NSPLIT, JT_PER_S, 128)
                .transpose(3, 0, 1, 5, 4, 2).astype(f16)),
        "wc": a(np.asarray(Wc, f).reshape(C, OT, 128).transpose(2, 1, 0)
                .astype(f16)),
        "b1s": a(np.asarray(b1, f).reshape(J // 128, 128).T),
        "cidx": a(cidx),
        "slotp1": a(slotp1),
    }
    if flags["bg"]:
        bgb = a(np.broadcast_to(np.asarray(bg, f).reshape(1, E), (128, E)))
        shared["bgb"] = bgb
        shared["bgbh"] = a(bgb.astype(bf))
    if flags["gb"]:
        shared["gammab"] = a(np.broadcast_to(np.asarray(gamma, f).reshape(1, E),
                                             (128, E)))
        shared["betab"] = a(np.broadcast_to(np.asarray(beta, f).reshape(1, E),
                                            (128, E)))
    if flags["b2"]:
        shared["b2s"] = a(np.asarray(b2, f).reshape(NDT, 128).T)
    if flags["bc"]:
        shared["bcr"] = a(np.asarray(bc, f).reshape(1, C).astype(f16))
    # gating column st*128+p must hold xs column p*4+st
    perm = (np.arange(T) % 128) * TS + np.arange(T) // 128
    in_maps = []
    for c in range(NCORES):
        xsh = a(x[c * T:(c + 1) * T].reshape(T, ITS, 128).transpose(2, 1, 0))
        m = dict(shared)
        m["xg"] = a(xsh[:, :, perm])
        m["xs"] = a(xsh.astype(f16))
        in_maps.append(m)
    return flags, in_maps


def _run(inputs, trace=False):
    flags, in_maps = _prep_inputs(**inputs)
    nc = _get_program(flags)
    res = run_bass_kernel_spmd(nc, in_maps, core_ids=list(range(NCORES)),
                               trace=trace)
    out = np.concatenate(
        [res.results[c]["out"].reshape(T, C) for c in range(NCORES)], axis=0)
    return out, res


def kernel(**inputs) -> np.ndarray:
    out, _ = _run(inputs, trace=False)
    return out


# revision 28
# speedup vs baseline: 1.1012x; 1.1012x over previous
"""MoE-GRN kernel for Trainium2, 8 NeuronCores, data-parallel over batch,
with on-device top-2 routing so fc2 only computes selected experts.

Reference (B=4096, IN=1024, J=HID*E=16384, Dtot=OUT*E=8192, E=8, C=1000, K=2):
    gate_logits = x @ Wg.T + bg                     [B, E]
    Gx = ||gate_logits||_2 per row; Nx = Gx / (mean_B(Gx) + 1e-6)
    gate_probs = softmax(gamma * (gate_logits * Nx) + beta)
    top2 over E=8
    h  = relu(x @ W1.T + b1)                        [B, J]
    eo = (h @ W2.T + b2).reshape(B, E, OUT)
    out = sum_k topk_probs * eo[topk_idx]           [B, OUT]
    y  = out @ Wc.T + bc                            [B, C]

Only 2 of 8 expert output blocks are needed per token, so the dense fc2
(B*J*Dtot MACs) is 4x oversized.  Per core (512 tokens): compute probs +
top-2 mask exactly (fp32 gating, min top2/top3 gap ~4e-5), compact each
expert's token list with gpsimd.sparse_gather (capacity 160 per expert,
observed max 153), ap_gather the fp16 h columns per expert per J-split,
run fc2 on just those columns, then weight by gate probs and scatter_add
back into a token-indexed buffer (bf16 hi+lo split keeps fp32-ish
precision through the bf16-only scatter path).

Token "slots": gating subtile st / partition p computes the token at xs
column p*4+st (host permutes the gating copy xg accordingly) so that the
(p, st) value grid folds into sparse_gather's 16-partition-wrapped stream
order with a pure strided DRAM readback.  Invalid (padding) list entries
are pointed at trash slot 512 whose gate weight is 0.

Precision: fc1/fc2 run in fp16 (weights + activations; moving-operand
fp16 keeps full PE rate at the 160-wide routed matmuls, and fp16's 11-bit
mantissa keeps the error well inside the 2e-2 gate where bf16 fails);
gating is fp32; combine/classifier fp16/f32.
"""

import numpy as np
import ml_dtypes

import concourse.bass as bass
import concourse.mybir as mybir
import concourse.tile as tile
from concourse import bacc
from concourse.bass_utils import run_bass_kernel_spmd

F32 = mybir.dt.float32
F32R = mybir.dt.float32r
F16 = mybir.dt.float16
BF16 = mybir.dt.bfloat16
I16 = mybir.dt.int16
U32 = mybir.dt.uint32
AF = mybir.ActivationFunctionType
ALU = mybir.AluOpType

B, IN, J, DTOT, E, C = 4096, 1024, 16384, 8192, 8, 1000
NCORES = 8
T = B // NCORES            # 512 tokens per core
TS = T // 128              # 4 token subtiles
ITS = IN // 128            # 8 k-subtiles over IN
NSPLIT = 8                 # J split into 8 chunks of 2048
JT_PER_S = J // NSPLIT // 128   # 16 j-tiles per split
OT = 1024 // 128           # 8 out-subtiles per expert block
NDT = DTOT // 128          # 64 d-tiles
NTT = B // 128             # 32 full-batch token tiles
CE = 160                   # routed capacity per (core, expert)
CW = CE // 16              # capacity in 16-wrapped columns
NSLOT = T + 1              # 512 real slots + trash slot 512
EPS = 1e-6


def _build(flags):
    has_bg, has_gb, has_b2, has_bc = (
        flags["bg"], flags["gb"], flags["b2"], flags["bc"])
    nc = bacc.Bacc("TRN2", target_bir_lowering=False)

    # ---- DRAM I/O ----
    xf_d = nc.dram_tensor("xf", [NTT, 128, ITS, 128], BF16, kind="ExternalInput")
    xg_d = nc.dram_tensor("xg", [128, ITS, T], F32, kind="ExternalInput")
    xs_d = nc.dram_tensor("xs", [128, ITS, T], F16, kind="ExternalInput")
    wg_d = nc.dram_tensor("wg", [128, ITS, E], F32, kind="ExternalInput")
    wgh_d = nc.dram_tensor("wgh", [128, ITS, E], BF16, kind="ExternalInput")
    w1_d = nc.dram_tensor("w1", [J // 128, 128, ITS, 128], F16, kind="ExternalInput")
    w2_d = nc.dram_tensor("w2", [NSPLIT, E, OT, 128, JT_PER_S, 128], F16,
                          kind="ExternalInput")
    wc_d = nc.dram_tensor("wc", [128, OT, C], F16, kind="ExternalInput")
    b1_d = nc.dram_tensor("b1s", [128, J // 128], F32, kind="ExternalInput")
    cidx_d = nc.dram_tensor("cidx", [16, CW], F32, kind="ExternalInput")
    slotp1_d = nc.dram_tensor("slotp1", [128, TS, E], F32, kind="ExternalInput")
    if has_bg:
        bg_d = nc.dram_tensor("bgb", [128, E], F32, kind="ExternalInput")
        bgh_d = nc.dram_tensor("bgbh", [128, E], BF16, kind="ExternalInput")
    if has_gb:
        ga_d = nc.dram_tensor("gammab", [128, E], F32, kind="ExternalInput")
        be_d = nc.dram_tensor("betab", [128, E], F32, kind="ExternalInput")
    if has_b2:
        b2_d = nc.dram_tensor("b2s", [128, NDT], F32, kind="ExternalInput")
    if has_bc:
        bc_d = nc.dram_tensor("bcr", [1, C], F16, kind="ExternalInput")
    out_d = nc.dram_tensor("out", [TS, 128, C], F32, kind="ExternalOutput")

    with tile.TileContext(nc) as tc:
        with tc.tile_pool(name="const", bufs=1) as cp, \
             tc.tile_pool(name="dram", bufs=1, space="DRAM") as dp, \
             tc.tile_pool(name="ps", bufs=6, space="PSUM") as psp:
            # resident tiles needed by fc1 first (their DMAs land first)
            xs = cp.tile([128, ITS, T], F16, tag="xs")
            nc.sync.dma_start(xs[:], xs_d[:])
            b1s = cp.tile([128, J // 128], F32, tag="b1s")
            nc.sync.dma_start(b1s[:], b1_d[:])
            split_cm = [tc.tile_pool(name="hqp", bufs=2),
                        tc.tile_pool(name="w1p", bufs=3),
                        tc.tile_pool(name="w2p", bufs=5),
                        tc.tile_pool(name="gthp", bufs=2)]
            hqp, w1p, w2p, gthp = [cm.__enter__() for cm in split_cm]

            def fc1_split(s):
                # hq token-major fp16 [slot, jtl]; slot 512 zeroed (trash)
                hq = hqp.tile([128, NSLOT, JT_PER_S], F16, tag="hq")
                nc.any.memset(hq[:, T, :], 0.0)
                for jtl in range(JT_PER_S):
                    jt = s * JT_PER_S + jtl
                    w1t = w1p.tile([128, ITS, 128], F16, tag="w1t")
                    nc.sync.dma_start(w1t[:], w1_d[jt])
                    ph = psp.tile([128, T], F32, tag="ps_shared")
                    for it in range(ITS):
                        nc.tensor.matmul(ph[:], w1t[:, it, :], xs[:, it, :],
                                         start=(it == 0), stop=(it == ITS - 1))
                    nc.scalar.activation(hq[:, 0:T, jtl], ph[:], AF.Relu,
                                         bias=b1s[:, jt:jt + 1])
                return hq

            # (fc1 split 0 is emitted from the orchestration block below,
            # after the shard gating + compaction that fc2 waits on)

            # ---- gating ----
            wg = cp.tile([128, ITS, E], F32, tag="wg")
            nc.sync.dma_start(wg[:], wg_d[:])
            wgh = cp.tile([128, ITS, E], BF16, tag="wgh")
            nc.sync.dma_start(wgh[:], wgh_d[:])
            cidx = cp.tile([16, CW], F32, tag="cidx")
            nc.sync.dma_start(cidx[:], cidx_d[:])
            slotp1 = cp.tile([128, TS, E], F32, tag="slotp1")
            nc.sync.dma_start(slotp1[:], slotp1_d[:])
            if has_bg:
                bgb = cp.tile([128, E], F32, tag="bgb")
                nc.sync.dma_start(bgb[:], bg_d[:])
                bgbh = cp.tile([128, E], BF16, tag="bgbh")
                nc.sync.dma_start(bgbh[:], bgh_d[:])
            if has_gb:
                gab = cp.tile([128, E], F32, tag="gammab")
                nc.sync.dma_start(gab[:], ga_d[:])
                beb = cp.tile([128, E], F32, tag="betab")
                nc.sync.dma_start(beb[:], be_d[:])
            ones1 = cp.tile([1, 128], F32, tag="ones1")
            nc.any.memset(ones1[:], 1.0)
            ones_c = cp.tile([128, 1], F32, tag="ones_c")
            nc.any.memset(ones_c[:], 1.0)

            # slot-indexed per-expert gate weights (0 for unselected/trash)
            wb = cp.tile([128, E, NSLOT], F32, tag="wb")
            nc.any.memset(wb[:, :, T], 0.0)
            # routed idx lists: per expert CE entries, 16-wrapped, int16.
            # idx512 variant points invalid entries at trash slot 512.
            idx128 = cp.tile([128, E * CW], I16, tag="idx128")
            wcol = cp.tile([128, E, CE], F32, tag="wcol")
            # fc2 accumulator over splits + final combine target
            eoac = cp.tile([128, E * CE, OT], F32, tag="eoac")
            moeh = cp.tile([128, NSLOT, OT], BF16, tag="moeh")
            nc.any.memset(moeh[:], 0.0)
            moel = cp.tile([128, NSLOT, OT], BF16, tag="moel")
            nc.any.memset(moel[:], 0.0)

            gating_cm = [tc.tile_pool(name="gxp", bufs=1),
                         tc.tile_pool(name="gin", bufs=2),
                         tc.tile_pool(name="gtmp", bufs=4),
                         tc.tile_pool(name="gps", bufs=2, space="PSUM")]
            gxp, gin, gt, gps = [cm.__enter__() for cm in gating_cm]
            xg = gxp.tile([128, ITS, T], F32, tag="xg")
            nc.sync.dma_start(xg[:], xg_d[:])

            ss_all = gxp.tile([128, NTT], F32, tag="ss_all")

            def xf_norm_tiles(tt0, tt1):
                # full-batch squared row norms of gate logits (bf16 inputs:
                # only feeds the batch mean, which averages the error away)
                for tt in range(tt0, tt1):
                    xt = gin.tile([128, ITS, 128], BF16, tag="xf_t")
                    nc.sync.dma_start(xt[:], xf_d[tt])
                    pg = gps.tile([128, E], F32, tag="pg")
                    for it in range(ITS):
                        nc.tensor.matmul(pg[:], xt[:, it, :], wgh[:, it, :],
                                         start=(it == 0), stop=(it == ITS - 1))
                    if has_bg:
                        lg = gt.tile([128, E], F32, tag="lg")
                        nc.vector.tensor_add(lg[:], pg[:], bgbh[:])
                        src = lg
                    else:
                        src = pg
                    sq = gt.tile([128, E], F32, tag="sq")
                    nc.scalar.square(sq[:], src[:])
                    nc.vector.reduce_sum(ss_all[:, tt:tt + 1], sq[:],
                                         axis=mybir.AxisListType.X)

            def mean_finish():
                gx_all = gt.tile([128, NTT], F32, tag="gx_all")
                nc.scalar.activation(gx_all[:], ss_all[:], AF.Sqrt)
                gsum = gt.tile([128, 1], F32, tag="gsum")
                nc.vector.reduce_sum(gsum[:], gx_all[:], axis=mybir.AxisListType.X)
                # partition-sum + mean + reciprocal + partition-broadcast, all
                # via tiny PE matmuls
                ptot = gps.tile([128, E], F32, tag="pg")
                nc.tensor.matmul(ptot[:1, :1], ones_c[:], gsum[:],
                                 start=True, stop=True)
                t1 = gt.tile([1, 1], F32, tag="t1")
                nc.vector.tensor_scalar(t1[:], ptot[:1, :1], 1.0 / B, EPS,
                                        op0=ALU.mult, op1=ALU.add)
                rec1 = gt.tile([1, 1], F32, tag="rec1")
                nc.vector.reciprocal(rec1[:], t1[:])
                pbc = gps.tile([128, E], F32, tag="pg")
                nc.tensor.matmul(pbc[:, :1], ones1[:], rec1[:],
                                 start=True, stop=True)
                nxs = gt.tile([128, 1], F32, tag="nxs")
                nc.scalar.copy(nxs[:], pbc[:, :1])
                return nxs

            # shard gating pass A (fp32, exact): GRN-mean-free modded logits
            # modp = lgs * ||lgs|| and the top-2 mask.  With gamma=1/beta=0
            # the batch mean only scales a token's logits by a positive
            # scalar, so top-2 selection (and hence compaction) does not
            # need the full-batch pass at all — that runs later, off the
            # critical path to the first routed fc2.
            # gating column st*128+p is slot p*4+st (host permutes xg).
            modp_all = gxp.tile([128, TS, E], F32, tag="modp")
            msk_all = gxp.tile([128, TS, E], F32, tag="mskall")
            w_all = gxp.tile([128, TS, E], F32, tag="w_all")

            def shard_passA(nxs):
                for st in range(TS):
                    pgs = gps.tile([128, E], F32, tag="pg")
                    for it in range(ITS):
                        nc.tensor.matmul(pgs[:],
                                         xg[:, it, st * 128:(st + 1) * 128],
                                         wg[:, it, :],
                                         start=(it == 0), stop=(it == ITS - 1))
                    lgs = gt.tile([128, E], F32, tag="lgs")
                    if has_bg:
                        nc.vector.tensor_add(lgs[:], pgs[:], bgb[:])
                    else:
                        nc.scalar.copy(lgs[:], pgs[:])
                    sq = gt.tile([128, E], F32, tag="sq")
                    nc.scalar.square(sq[:], lgs[:])
                    ss1 = gt.tile([128, 1], F32, tag="ss1")
                    nc.vector.reduce_sum(ss1[:], sq[:], axis=mybir.AxisListType.X)
                    gx1 = gt.tile([128, 1], F32, tag="gx1")
                    nc.scalar.activation(gx1[:], ss1[:], AF.Sqrt)
                    if nxs is not None:
                        nx = gt.tile([128, 1], F32, tag="nx")
                        nc.vector.tensor_mul(nx[:], gx1[:], nxs[:])
                        gx1 = nx
                    mod = modp_all[:, st, :]
                    nc.vector.tensor_scalar_mul(mod, lgs[:], gx1[:])
                    if has_gb:
                        nc.vector.tensor_mul(mod, mod, gab[:])
                        nc.vector.tensor_add(mod, mod, beb[:])
                    mx8 = gt.tile([128, 8], F32, tag="mx8")
                    nc.vector.max(mx8[:], mod)
                    nc.vector.tensor_scalar(msk_all[:, st, :], mod,
                                            mx8[:, 1:2], None, op0=ALU.is_ge)

            def weight_passB(nxs):
                # softmax weights from the (now mean-scaled) logits
                for st in range(TS):
                    mod = gt.tile([128, E], F32, tag="mod")
                    if nxs is not None:
                        nc.vector.tensor_scalar_mul(mod[:], modp_all[:, st, :],
                                                    nxs[:])
                    else:
                        nc.scalar.copy(mod[:], modp_all[:, st, :])
                    rmax = gt.tile([128, 1], F32, tag="rmax")
                    nc.vector.reduce_max(rmax[:], mod[:],
                                         axis=mybir.AxisListType.X)
                    nrm = gt.tile([128, 1], F32, tag="nrm")
                    nc.vector.tensor_scalar_mul(nrm[:], rmax[:], -1.0)
                    ex = gt.tile([128, E], F32, tag="ex")
                    nc.scalar.activation(ex[:], mod[:], AF.Exp, bias=nrm[:])
                    sm = gt.tile([128, 1], F32, tag="sm")
                    nc.vector.reduce_sum(sm[:], ex[:], axis=mybir.AxisListType.X)
                    rs = gt.tile([128, 1], F32, tag="rs")
                    nc.vector.reciprocal(rs[:], sm[:])
                    probs = gt.tile([128, E], F32, tag="probs")
                    nc.vector.tensor_scalar_mul(probs[:], ex[:], rs[:])
                    nc.vector.tensor_mul(w_all[:, st, :], msk_all[:, st, :],
                                         probs[:])

            def compaction():
                # val[p, st, e] = slot (p*4+st) if expert selected else -1
                m01 = msk_all
                val = gxp.tile([128, TS, E], F32, tag="val")
                nc.vector.tensor_mul(val[:], m01[:], slotp1[:])
                nc.vector.tensor_scalar(val[:], val[:], -1.0, None,
                                        op0=ALU.add)
                # bounce val through DRAM to reach the slot-major 16-wrap
                # layout (SWDGE queue, off the big DMA rings)
                vdr = dp.tile([128, TS, E], F32, tag="vdr")
                nc.gpsimd.dma_start(vdr[:], val[:])
                vread = vdr[:].rearrange("(a b) s e -> e (b s) a", b=TS)
                for e in range(E):
                    vin = gin.tile([16, T // 16], F32, tag="vin")
                    nc.gpsimd.dma_start(vin[:], vread[e])
                    sg = gin.tile([16, CW], F32, tag="sg")
                    nf = gin.tile([1, 1], U32, tag="nf")
                    nc.gpsimd.sparse_gather(sg[:], vin[:], num_found=nf[:])
                    nff = gt.tile([1, 1], F32, tag="nff")
                    nc.vector.tensor_copy(nff[:], nf[:])
                    nf128 = gt.tile([128, 1], F32, tag="nf128")
                    nc.gpsimd.partition_broadcast(nf128[:], nff[:])
                    vmask = gt.tile([16, CW], F32, tag="vmask")
                    nc.vector.tensor_tensor(
                        vmask[:], cidx[:], nf128[0:16, :].to_broadcast([16, CW]),
                        op=ALU.is_lt)
                    # invalid entries -> trash slot 512, branchlessly:
                    # clamp(sg,0,T), then vmask*(sgc-T)+T
                    sgc = gt.tile([16, CW], F32, tag="sgc")
                    nc.vector.tensor_scalar(sgc[:], sg[:], 0.0, float(T),
                                            op0=ALU.max, op1=ALU.min)
                    nc.vector.tensor_scalar(sgc[:], sgc[:], float(T), None,
                                            op0=ALU.subtract)
                    sfix = gt.tile([16, CW], F32, tag="sfix")
                    nc.vector.tensor_tensor(sfix[:], vmask[:], sgc[:],
                                            op=ALU.mult)
                    nc.vector.tensor_scalar(sfix[:], sfix[:], float(T), None,
                                            op0=ALU.add)
                    nc.vector.tensor_copy(idx128[0:16, e * CW:(e + 1) * CW],
                                          sfix[:])
                # replicate idx lists to all 8 16-partition groups
                for g in range(1, 8):
                    nc.gpsimd.dma_start(idx128[16 * g:16 * g + 16, :],
                                        idx128[0:16, :])

            def weight_tables():
                # w_all -> DRAM bounce -> slot-major wb -> routed wcol
                wdr = dp.tile([E, TS, 128], F32, tag="wdr")
                for st in range(TS):
                    nc.gpsimd.dma_start(wdr[:, st, :].rearrange("e p -> p e"),
                                        w_all[:, st, :])
                # wb[*, e, slot] with slot = p*4+st  <=  wdr[e, st, p]
                wrows = gxp.tile([1, E, T], F32, tag="wrows")
                for e in range(E):
                    nc.gpsimd.dma_start(
                        wrows[:, e, :].rearrange("o (p s) -> o p s", s=TS),
                        wdr[e].rearrange("s p -> p s")[None])
                    nc.gpsimd.partition_broadcast(wb[:, e, 0:T], wrows[:, e, :])
                for e in range(E):
                    nc.gpsimd.ap_gather(
                        wcol[:, e, :], wb[:, e, :],
                        idx128[:, e * CW:(e + 1) * CW],
                        channels=128, num_elems=NSLOT, d=1, num_idxs=CE)

            # ---- fc2: routed per-expert matmuls, accumulated over splits ----
            if has_b2:
                b2s = cp.tile([128, NDT], F32, tag="b2s")
                nc.sync.dma_start(b2s[:], b2_d[:])

            def fc2_split(s, hq):
                for e in range(E):
                    gth = gthp.tile([128, CE, JT_PER_S], F16, tag="gth")
                    nc.gpsimd.ap_gather(
                        gth[:], hq[:],
                        idx128[:, e * CW:(e + 1) * CW],
                        channels=128, num_elems=NSLOT, d=JT_PER_S,
                        num_idxs=CE)
                    # transpose to kt-major so every fc2 matmul reads a
                    # CONTIGUOUS moving operand (strided rhs starves the PE)
                    gtt = gthp.tile([128, JT_PER_S, CE], F16, tag="gtt")
                    nc.vector.tensor_copy(gtt[:],
                                          gth[:].rearrange("p c k -> p k c"))
                    for ot in range(OT):
                        w2t = w2p.tile([128, JT_PER_S, 128], F16, tag="w2t")
                        nc.sync.dma_start(w2t[:], w2_d[s, e, ot])
                        pe_ = psp.tile([128, CE], F32, tag="ps_shared")
                        for ktl in range(JT_PER_S):
                            nc.tensor.matmul(pe_[:], w2t[:, ktl, :],
                                             gtt[:, ktl, :],
                                             start=(ktl == 0),
                                             stop=(ktl == JT_PER_S - 1))
                        seg = eoac[:, e * CE:(e + 1) * CE, ot]
                        if s == 0:
                            if has_b2:
                                nc.scalar.activation(
                                    seg, pe_[:], AF.Identity,
                                    bias=b2s[:, e * OT + ot:e * OT + ot + 1])
                            else:
                                nc.scalar.copy(seg, pe_[:])
                        else:
                            nc.vector.tensor_add(seg, seg, pe_[:])

            # ---- orchestration ----
            if has_gb:
                # gamma/beta break the scale-invariance of top-2: the mean
                # must precede mask computation (original ordering)
                hq0 = fc1_split(0)
                xf_norm_tiles(0, NTT)
                nxs = mean_finish()
                shard_passA(nxs)
                compaction()
                weight_passB(None)
                weight_tables()
                hq_prev = fc1_split(1)
                fc2_split(0, hq0)
            else:
                # masks don't depend on the batch mean: compact immediately
                # (before even fc1) so fc2 starts draining the W2 stream as
                # early as possible; the full-batch mean pass (only needed
                # for the combine weights) is emitted in two chunks sized to
                # the per-split PE slack so the W2 stream never stalls.
                shard_passA(None)
                compaction()
                hq0 = fc1_split(0)
                hq_prev = fc1_split(1)
                fc2_split(0, hq0)
                xf_norm_tiles(0, NTT // 2)
                hq = fc1_split(2)
                fc2_split(1, hq_prev)
                hq_prev = hq
                xf_norm_tiles(NTT // 2, NTT)
                nxs = mean_finish()
                weight_passB(nxs)
                weight_tables()
            for cm in reversed(gating_cm):
                cm.__exit__(None, None, None)

            clp_cm = tc.tile_pool(name="clsp", bufs=1)
            clp = None
            s0 = 2 if has_gb else 3
            for s in range(s0, NSPLIT + 1):
                if s == NSPLIT - 1:
                    # prefetch classifier weights behind the last split's w2
                    clp = clp_cm.__enter__()
                    wc = clp.tile([128, OT, C], F16, tag="wc")
                    nc.sync.dma_start(wc[:], wc_d[:])
                    if has_bc:
                        bct = clp.tile([1, C], F16, tag="bcr")
                        nc.sync.dma_start(bct[:], bc_d[:])
                        ones1h = clp.tile([1, 128], F16, tag="ones1h")
                        nc.any.memset(ones1h[:], 1.0)
                if s < NSPLIT:
                    hq = fc1_split(s)
                    fc2_split(s - 1, hq_prev)
                    hq_prev = hq
                else:
                    fc2_split(NSPLIT - 1, hq_prev)

            # ---- combine: weight by gate prob, bf16 hi/lo scatter-add ----
            with tc.tile_pool(name="cmb", bufs=1) as cmb:
                for e in range(E):
                    eow = cmb.tile([128, CE, OT], F32, tag="eow")
                    nc.vector.tensor_tensor(
                        eow[:], eoac[:, e * CE:(e + 1) * CE, :],
                        wcol[:, e, :].rearrange("p (n u) -> p n u", u=1)
                        .to_broadcast([128, CE, OT]),
                        op=ALU.mult)
                    ehi = cmb.tile([128, CE, OT], BF16, tag="ehi")
                    nc.vector.tensor_copy(ehi[:], eow[:])
                    elo = cmb.tile([128, CE, OT], BF16, tag="elo")
                    nc.vector.tensor_tensor(elo[:], eow[:], ehi[:],
                                            op=ALU.subtract)
                    nc.gpsimd.scatter_add(
                        moeh[:], idx128[:, e * CW:(e + 1) * CW], ehi[:],
                        channels=128, num_elems=NSLOT, d=OT, num_idxs=CE)
                    nc.gpsimd.scatter_add(
                        moel[:], idx128[:, e * CW:(e + 1) * CW], elo[:],
                        channels=128, num_elems=NSLOT, d=OT, num_idxs=CE)
                moe16 = clp.tile([128, NSLOT, OT], F16, tag="moe16")
                nc.vector.tensor_add(moe16[:], moeh[:], moel[:])

            # ---- classifier (fp16 x fp16) ----
            with tc.tile_pool(name="outp", bufs=2) as outp:
                for st in range(TS):
                    ot_ = outp.tile([128, C], F32, tag="ot")
                    for c0, cw_ in ((0, 512), (512, C - 512)):
                        pc = psp.tile([128, T], F32, tag="ps_shared")
                        for kt in range(OT):
                            nc.tensor.matmul(
                                pc[:, :cw_],
                                moe16[:, st * 128:(st + 1) * 128, kt],
                                wc[:, kt, c0:c0 + cw_],
                                start=(kt == 0),
                                stop=(kt == OT - 1 and not has_bc))
                        if has_bc:
                            nc.tensor.matmul(pc[:, :cw_], ones1h[:],
                                             bct[:, c0:c0 + cw_],
                                             start=False, stop=True)
                        nc.scalar.copy(ot_[:, c0:c0 + cw_], pc[:, :cw_])
                    nc.sync.dma_start(out_d[st], ot_[:])
            clp_cm.__exit__(None, None, None)
            for cm in reversed(split_cm):
                cm.__exit__(None, None, None)

    nc.compile()
    return nc


_CACHE = {}


def _get_program(flags):
    key = tuple(sorted(flags.items()))
    if key not in _CACHE:
        _CACHE[key] = _build(flags)
    return _CACHE[key]


def _prep_inputs(x, Wg, bg, gamma, beta, W1, b1, W2, b2, Wc, bc):
    f = np.float32
    bf = ml_dtypes.bfloat16
    f16 = np.float16
    a = np.ascontiguousarray
    x = np.asarray(x, f)
    flags = {
        "bg": bool(np.any(np.asarray(bg))),
        "gb": bool(np.any(np.asarray(gamma) != 1.0) or np.any(np.asarray(beta))),
        "b2": bool(np.any(np.asarray(b2))),
        "bc": bool(np.any(np.asarray(bc))),
    }
    wg_t = np.asarray(Wg, f).reshape(E, ITS, 128).transpose(2, 1, 0)
    # constants for on-device compaction
    cidx = (np.arange(CW)[None, :] * 16 + np.arange(16)[:, None]).astype(f)
    slotp1 = (np.arange(128)[:, None] * TS + np.arange(TS)[None, :] + 1.0)
    slotp1 = np.broadcast_to(slotp1[:, :, None], (128, TS, E)).astype(f)
    shared = {
        "xf": a(x.reshape(NTT, 128, ITS, 128).transpose(0, 3, 2, 1)
                .astype(bf)),
        "wg": a(wg_t),
        "wgh": a(wg_t.astype(bf)),
        "w1": a(np.asarray(W1, f).reshape(J // 128, 128, ITS, 128)
                .transpose(0, 3, 2, 1).astype(f16)),
        "w2": a(np.asarray(W2, f).reshape(E, OT, 128, NSPLIT, JT_PER_S, 128)
                .transpose(3, 0, 1, 5, 4, 2).astype(f16)),
        "wc": a(np.asarray(Wc, f).reshape(C, OT, 128).transpose(2, 1, 0)
                .astype(f16)),
        "b1s": a(np.asarray(b1, f).reshape(J // 128, 128).T),
        "cidx": a(cidx),
        "slotp1": a(slotp1),
    }
    if flags["bg"]:
        bgb = a(np.broadcast_to(np.asarray(bg, f).reshape(1, E), (128, E)))
        shared["bgb"] = bgb
        shared["bgbh"] = a(bgb.astype(bf))
    if flags["gb"]:
        shared["gammab"] = a(np.broadcast_to(np.asarray(gamma, f).reshape(1, E),
                                             (128, E)))
        shared["betab"] = a(np.broadcast_to(np.asarray(beta, f).reshape(1, E),
                                            (128, E)))
    if flags["b2"]:
        shared["b2s"] = a(np.asarray(b2, f).reshape(NDT, 128).T)
    if flags["bc"]:
        shared["bcr"] = a(np.asarray(bc, f).reshape(1, C).astype(f16))
    # gating column st*128+p must hold xs column p*4+st
    perm = (np.arange(T) % 128) * TS + np.arange(T) // 128
    in_maps = []
    for c in range(NCORES):
        xsh = a(x[c * T:(c + 1) * T].reshape(T, ITS, 128).transpose(2, 1, 0))
        m = dict(shared)
        m["xg"] = a(xsh[:, :, perm])
        m["xs"] = a(xsh.astype(f16))
        in_maps.append(m)
    return flags, in_maps


def _run(inputs, trace=False):
    flags, in_maps = _prep_inputs(**inputs)
    nc = _get_program(flags)
    res = run_bass_kernel_spmd(nc, in_maps, core_ids=list(range(NCORES)),
                               trace=trace)
    out = np.concatenate(
        [res.results[c]["out"].reshape(T, C) for c in range(NCORES)], axis=0)
    return out, res


def kernel(**inputs) -> np.ndarray:
    out, _ = _run(inputs, trace=False)
    return out
